# revision 9
# baseline (speedup 1.0000x reference)
"""Causal self-attention Trainium2 Bass kernel (8 NeuronCores).

Problem: B=2, T=4096, C=512, H=8 heads, D=64 head dim.
  qkv = x @ w_attn.T + b_attn ; causal softmax attention ; y @ w_proj.T + b_proj

Sharding: 16 (batch, head) units over 8 cores -> each core handles one batch
and two adjacent heads (core = b*4 + hp, heads 2hp and 2hp+1). Weights are
sliced per core on the host; each core computes a [C, T] partial of the
output projection for its batch (heads contribution); host sums the 4
partials per batch and transposes back.

v2 (this file): all matmul operands in bf16 (1 cycle/col on the PE at any
free size, vs fp32r's 4-byte stream; also halves SBUF/DMA traffic and PE
power -> less HAM util-throttle). Attention loops i-outer with adjacent
J-blocks paired into one [128,1024] PSUM tile so each exp covers 1024 cols
(amortizes the ~220-cycle ACT per-instruction bubble). Elementwise work is
spread over DVE (bias adds, masks, reciprocal, divisions) and Pool (V
copies, proj bias) so the ACT engine only runs exp.

On-device layout is fully transposed (dims x T):
  qT/kT [128, T] bf16 (2 heads x 64 dims stacked), scores S.T [j, i] per
  (128-key-block x 512-query-tile), softmax denominator via an appended
  ones-column on the V operand of the P@V matmul, unsafe softmax (no max
  subtraction; scores are O(N(0,1)) so exp never overflows bf16), division
  by the denominator after each i-tile's J loop (DRAM-bounce partition
  broadcast + reciprocal), projection emitted as out.T [C, T] fp32.
"""

import numpy as np

import concourse.bacc as bacc
import concourse.tile as tile
import concourse.mybir as mybir
from concourse import bass_utils
from concourse.bass import AP

F32 = mybir.dt.float32
BF16 = mybir.dt.bfloat16
AF = mybir.ActivationFunctionType

B, T, C = 2, 4096, 512
H, D = 8, 64
N_CORES = 8
TQ = 512          # query tile (i-tile)
TJ = 128          # key block (j-block)
NI = T // TQ      # 8 i-tiles
NJ = T // TJ      # 32 j-blocks

MM_DT = BF16


def _emit(nc, tc, ctx):
    xT = nc.dram_tensor("xT", [C, T], MM_DT, kind="ExternalInput").ap()
    wqkvT = nc.dram_tensor("wqkvT", [C, 384], MM_DT, kind="ExternalInput").ap()
    bqkv = nc.dram_tensor("bqkv", [128, 3], F32, kind="ExternalInput").ap()
    wpT = nc.dram_tensor("wpT", [128, C], MM_DT, kind="ExternalInput").ap()
    bp = nc.dram_tensor("bp", [128, 4], F32, kind="ExternalInput").ap()
    mask01 = nc.dram_tensor("mask01", [128, 128], MM_DT, kind="ExternalInput").ap()
    ident = nc.dram_tensor("ident", [128, 128], MM_DT, kind="ExternalInput").ap()
    outT = nc.dram_tensor("outT", [C, T], F32, kind="ExternalOutput").ap()

    consts = ctx.enter_context(tc.tile_pool(name="consts", bufs=1))
    big = ctx.enter_context(tc.tile_pool(name="big", bufs=1))
    xt_pool = ctx.enter_context(tc.tile_pool(name="xt", bufs=12))
    vt_pool = ctx.enter_context(tc.tile_pool(name="vt", bufs=4))
    pt_pool = ctx.enter_context(tc.tile_pool(name="pp", bufs=8))
    yn_pool = ctx.enter_context(tc.tile_pool(name="yn", bufs=8))
    dt_pool = ctx.enter_context(tc.tile_pool(name="dtp", bufs=4))
    dscr_pool = ctx.enter_context(tc.tile_pool(name="dscr", bufs=2, space="DRAM"))
    os_pool = ctx.enter_context(tc.tile_pool(name="osp", bufs=4))
    # PSUM: 8 banks. ps_a: 2 x [128,1024] f32 (2 banks each) for paired S
    # tiles (also holds the small bf16 V-transpose tiles in phase A).
    # ps_d: 2 x [128,512] (diagonal S tiles, QKV chunks, out-proj).
    # ps_o: 2 x [128,512] (the O' accumulator of the current (h, i_t)).
    ps_a = ctx.enter_context(tc.tile_pool(name="ps_a", bufs=2, space="PSUM"))
    ps_d = ctx.enter_context(tc.tile_pool(name="ps_d", bufs=2, space="PSUM"))
    ps_o = ctx.enter_context(tc.tile_pool(name="ps_o", bufs=2, space="PSUM"))

    # --- constants ---
    w_sb = consts.tile([128, 4, 384], MM_DT, name="w_sb")
    nc.sync.dma_start(out=w_sb, in_=wqkvT.rearrange("(c p) m -> p c m", p=128))
    # split per head so each lhsT sits at base partition 0 (matmul requires
    # lhsT and rhs to share the base partition; the rhs y tiles are at 0)
    wpm_sb = consts.tile([128, C], MM_DT, name="wpm_sb")
    nc.sync.dma_start(out=wpm_sb, in_=wpT)
    bqkv_sb = consts.tile([128, 3], F32, name="bqkv_sb")
    nc.sync.dma_start(out=bqkv_sb, in_=bqkv)
    bp_sb = consts.tile([128, 4], F32, name="bp_sb")
    nc.sync.dma_start(out=bp_sb, in_=bp)
    mask_sb = consts.tile([128, 128], MM_DT, name="mask_sb")
    nc.sync.dma_start(out=mask_sb, in_=mask01)
    id_sb = consts.tile([128, 128], MM_DT, name="id_sb")
    nc.sync.dma_start(out=id_sb, in_=ident)

    qT_sb = big.tile([128, T], MM_DT, name="qT_sb")
    kT_sb = big.tile([128, T], MM_DT, name="kT_sb")
    # V in natural layout per 128-key block, with a ones column appended per
    # head: [j, 0:64]=v_h0, 64=ones, [65:129]=v_h1, 129=ones.
    v_all = big.tile([128, NJ, 130], MM_DT, name="v_all")
    nc.gpsimd.memset(v_all[:, :, 64:65], 1.0)
    nc.gpsimd.memset(v_all[:, :, 129:130], 1.0)

    # --- QKV projection (transposed layout) ---
    for t in range(NI):
        t0 = t * TQ
        xcs = []
        for c in range(4):
            xc = xt_pool.tile([128, TQ], MM_DT, name="xc", tag="xc")
            deng = nc.sync if c % 2 == 0 else nc.gpsimd
            deng.dma_start(out=xc, in_=xT[c * 128:(c + 1) * 128, t0:t0 + TQ])
            xcs.append(xc)
        for m in range(3):  # q, k, v rows of the sliced w_attn
            ps = ps_d.tile([128, TQ], F32, name="qkv_ps", tag="d")
            for c in range(4):
                nc.tensor.matmul(
                    ps,
                    lhsT=w_sb[:, c, m * 128:(m + 1) * 128],
                    rhs=xcs[c],
                    start=(c == 0),
                    stop=(c == 3),
                )
            if m == 0:
                # q scale (1/sqrt(D)) is folded into wqkvT/bqkv on the host
                nc.scalar.add(qT_sb[:, t0:t0 + TQ], ps, bqkv_sb[:, 0:1])
            elif m == 1:
                nc.scalar.add(kT_sb[:, t0:t0 + TQ], ps, bqkv_sb[:, 1:2])
            else:
                vt = vt_pool.tile([128, TQ], MM_DT, name="vt", tag="vt")
                nc.scalar.add(vt, ps, bqkv_sb[:, 2:3])
                for s in range(4):
                    n = t * 4 + s
                    tp = ps_a.tile([128, 128], MM_DT, name="tp", tag="a")
                    # bf16 PE transpose (exact move): [vdim, t]^T -> [t, vdim]
                    nc.tensor.transpose(tp, vt[:, s * 128:(s + 1) * 128], id_sb)
                    dst = v_all[:, n, :].rearrange("p (g e) -> p g e", g=2, e=65)[:, :, 0:64]
                    src = tp.rearrange("p (g e) -> p g e", g=2, e=64)
                    nc.vector.tensor_copy(dst, src)

    # --- attention (i-outer; non-diagonal J-blocks paired per exp) ---
    yns = {}
    for half in range(2):
        i_ts = [half * 4 + k for k in range(4)]
        for h in range(2):
            hr = slice(h * 64, (h + 1) * 64)
            for i_t in i_ts:
                i0 = i_t * TQ
                njd = i_t * 4  # non-diagonal J-blocks for this i-tile
                o = ps_o.tile([128, TQ], F32, name="o_ps", tag="o")
                # diagonal blocks: S + exp + mask emitted first so their
                # 3-engine chains resolve while the pair pipeline runs;
                # their PV matmuls are appended after the pairs.
                pt_ds = []
                for s in range(4):
                    J = njd + s
                    j0 = J * TJ
                    r = s * TJ
                    st = ps_d.tile([128, TQ], F32, name="st", tag="d")
                    nc.tensor.matmul(
                        st[:, r:TQ],
                        lhsT=kT_sb[hr, j0:j0 + TJ],
                        rhs=qT_sb[hr, i0 + r:i0 + TQ],
                        start=True, stop=True,
                    )
                    pt = pt_pool.tile([128, TQ], MM_DT, name="ptd", tag="p")
                    nc.scalar.activation(pt[:, r:TQ], st[:, r:TQ], AF.Exp)
                    nc.vector.tensor_mul(pt[:, r:r + 128], pt[:, r:r + 128], mask_sb)
                    pt_ds.append((J, r, pt))
                for Jp in range(0, njd, 2):
                    st2 = ps_a.tile([128, 2 * TQ], F32, name="st2", tag="a")
                    for u in range(2):
                        j0 = (Jp + u) * TJ
                        nc.tensor.matmul(
                            st2[:, u * TQ:(u + 1) * TQ],
                            lhsT=kT_sb[hr, j0:j0 + TJ],
                            rhs=qT_sb[hr, i0:i0 + TQ],
                            start=True, stop=True,
                        )
                    pt = pt_pool.tile([128, 2 * TQ], MM_DT, name="pt", tag="p")
                    nc.scalar.activation(pt, st2, AF.Exp)
                    for u in range(2):
                        J = Jp + u
                        v_lhs = v_all[:, J, 0:65] if h == 0 else v_all[:, J, 65:130]
                        nc.tensor.matmul(
                            o[0:65, :],
                            lhsT=v_lhs,
                            rhs=pt[:, u * TQ:(u + 1) * TQ],
                            start=(J == 0),
                            stop=False,
                        )
                for s in range(4):  # diagonal PV, after the pair pipeline
                    J, r, pt = pt_ds[s]
                    v_lhs = v_all[:, J, 0:65] if h == 0 else v_all[:, J, 65:130]
                    nc.tensor.matmul(
                        o[0:65, r:TQ],
                        lhsT=v_lhs,
                        rhs=pt[:, r:TQ],
                        start=(J == 0),
                        stop=(s == 3),
                    )
                # softmax division: denominator (ones-column row, partition
                # 64) -> DRAM-bounce broadcast to partitions 0:64 ->
                # reciprocal (recip_approx_fast and partition_broadcast are
                # HW-broken at base partition != 0). h0 lands in ynm[0:64];
                # h1 computed at base 0 then DMA-moved to ynm[64:128] so the
                # out-projection runs as one K=128 matmul per column block.
                dsb = dt_pool.tile([65, TQ], F32, name="dsb", tag="dt")
                nc.vector.tensor_copy(dsb[64:65, :], o[64:65, :])
                scr = dscr_pool.tile([1, TQ], F32, name="scr", tag="scr")
                nc.gpsimd.dma_start(out=scr, in_=dsb[64:65, :])
                rep = dt_pool.tile([64, TQ], F32, name="rep", tag="dt")
                bc = AP(tensor=scr.tensor, offset=scr.offset, ap=[[0, 64], [1, TQ]])
                nc.gpsimd.dma_start(out=rep, in_=bc)
                rc = dt_pool.tile([64, TQ], F32, name="rc", tag="dt")
                nc.vector.reciprocal_approx_fast(out=rc, in_=rep)
                if h == 0:
                    ynm = yn_pool.tile([128, TQ], MM_DT, name="ynm", tag="ynm")
                    yns[i_t] = ynm
                    nc.vector.tensor_mul(ynm[0:64, :], o[0:64, :], rc)
                else:
                    y1 = vt_pool.tile([64, TQ], MM_DT, name="y1", tag="y1")
                    nc.vector.tensor_mul(y1, o[0:64, :], rc)
                    nc.gpsimd.dma_start(out=yns[i_t][64:128, :], in_=y1)
        # --- output projection for this half (partial out.T, 2 heads) ---
        for i_t in i_ts:
            i0 = i_t * TQ
            for mc in range(4):
                po = ps_d.tile([128, TQ], F32, name="po", tag="d")
                nc.tensor.matmul(po, lhsT=wpm_sb[:, mc * 128:(mc + 1) * 128],
                                 rhs=yns[i_t], start=True, stop=True)
                ob = os_pool.tile([128, TQ], F32, name="ob", tag="os")
                nc.vector.tensor_scalar_add(ob, po, bp_sb[:, mc:mc + 1])
                nc.sync.dma_start(out=outT[mc * 128:(mc + 1) * 128, i0:i0 + TQ], in_=ob)


_CACHED_NC = None


def _build_program():
    global _CACHED_NC
    if _CACHED_NC is not None:
        return _CACHED_NC
    from contextlib import ExitStack
    nc = bacc.Bacc("TRN2", target_bir_lowering=False, debug=False,
                   num_devices=N_CORES)
    with tile.TileContext(nc) as tc:
        with ExitStack() as ctx:
            _emit(nc, tc, ctx)
    nc.compile()
    _CACHED_NC = nc
    return nc


def _host_inputs(x, w_attn, b_attn, w_proj, b_proj):
    """Build the 8 per-core input maps."""
    import ml_dtypes
    mmnp = ml_dtypes.bfloat16 if MM_DT == BF16 else np.float32
    x = np.asarray(x, dtype=np.float32)
    w_attn = np.asarray(w_attn, dtype=np.float32)
    b_attn = np.asarray(b_attn, dtype=np.float32)
    w_proj = np.asarray(w_proj, dtype=np.float32)
    b_proj = np.asarray(b_proj, dtype=np.float32)

    scale = np.float32(1.0 / np.sqrt(D))
    mask = np.triu(np.ones((128, 128), dtype=np.float32))  # keep jj <= ii
    ident = np.eye(128, dtype=np.float32)

    xT_b = [np.ascontiguousarray(x[b].T).astype(mmnp) for b in range(B)]

    in_maps = []
    for core in range(N_CORES):
        b, hp = divmod(core, 4)
        r0 = 2 * hp * 64  # first row of this core's head-pair slice
        qr = w_attn[r0:r0 + 128] * scale
        kr = w_attn[C + r0:C + r0 + 128]
        vr = w_attn[2 * C + r0:2 * C + r0 + 128]
        wqkvT = np.ascontiguousarray(np.concatenate([qr, kr, vr], axis=0).T)
        bq = b_attn[r0:r0 + 128] * scale
        bk = b_attn[C + r0:C + r0 + 128]
        bv = b_attn[2 * C + r0:2 * C + r0 + 128]
        bqkv = np.ascontiguousarray(np.stack([bq, bk, bv], axis=1))
        wpT = np.ascontiguousarray(w_proj[:, r0:r0 + 128].T)
        if hp == 0:
            bp = np.ascontiguousarray(b_proj.reshape(4, 128).T)
        else:
            bp = np.zeros((128, 4), dtype=np.float32)
        in_maps.append({
            "xT": xT_b[b],
            "wqkvT": wqkvT.astype(mmnp),
            "bqkv": bqkv,
            "wpT": wpT.astype(mmnp),
            "bp": bp,
            "mask01": mask.astype(mmnp),
            "ident": ident.astype(mmnp),
        })
    return in_maps


def _gather(results):
    out = np.empty((B, T, C), dtype=np.float32)
    for b in range(B):
        acc = results[b * 4]["outT"].astype(np.float32).copy()
        for hp in range(1, 4):
            acc += results[b * 4 + hp]["outT"]
        out[b] = acc.T
    return out


def kernel(x, w_attn, b_attn, w_proj, b_proj, _run_kwargs=None):
    nc = _build_program()
    in_maps = _host_inputs(x, w_attn, b_attn, w_proj, b_proj)
    kw = dict(_run_kwargs or {})
    res = bass_utils.run_bass_kernel_spmd(nc, in_maps,
                                          core_ids=list(range(N_CORES)), **kw)
    out = _gather(res.results)
    if _run_kwargs is not None:
        kernel.last_result = res
    return out
